# revision 12
# baseline (speedup 1.0000x reference)
"""Trainium2 Bass kernel for nn_ConvLayer (gnn_message_passing).

Math: out[b,k,n] = sum_{m,j} W[b,n,m,j] * z_j[b,m,k] + bias[k]
  where z_j[b,m,k] = sum_c x[b,m,c] * conv_w[k, j*C+c]
(the reference's gmul + 1x1-conv collapses into one big GEMM against the
tiny precomputed z, streaming W exactly once -> memory-bound).

Sharding: 8 cores = 2 batches x 4 row-blocks of W's first node axis n.
Each core handles W[b, n0:n0+1024] (48 MiB), computes out[b, :, n0:n0+1024].
The shard handed to each core is laid out m-major on the host side
(WT[m, j*1024 + n] = W[b, n0+n, m, j]) so the contraction axis m lands on
SBUF partitions directly from the DMA; x / conv_w are likewise uploaded
pre-transposed (c on partitions).  These are pure permutations of the same
fp32 bytes -- all FLOPs (z build + main GEMM + bias) happen on-device and
W is still read from HBM in full fp32 exactly once.

Per-core device program (identical SPMD program, different data):
  Phase A: z build fully on-PE: pz[m-tile] = xT_tile^T @ cwT for all
    (j,k) at once -> Z (128, 32*96) f32r.  Constant loads (xT, cwT, bias)
    go through the GPSIMD (SWDGE) queue so the two HWDGE rings carry
    nothing but W traffic from t=0.
  Phase B: stream WT in 14 DMAs of 3 MiB (2 m-tiles each) + 4 tail DMAs
    of 1.5 MiB, alternating the two HWDGE rings (sync / scalar); 12 KiB
    contiguous per (partition, m-tile) descriptors run at full fabric
    rate (~420 GB/s/core).  For each m-tile q and operator j, two
    accumulating f32r matmuls (stationary zq [128,32], moving WT slice
    [128,512]) add into the two PSUM accumulator banks.  W passes through
    the PE exactly once as the 512-wide moving operand (~45 us PE vs
    ~125 us DMA -> purely DMA-bound).
  Phase C: bias + writeout -> DRAM as (32, 1024) = final (k, n) layout.
"""
import sys

if "/opt/trn_rl_repo" not in sys.path:
    sys.path.insert(0, "/opt/trn_rl_repo")

import numpy as np

import concourse.bacc as bacc
import concourse.mybir as mybir
from concourse import tile
from concourse.bass_utils import run_bass_kernel_spmd

dt = mybir.dt
F32 = dt.float32
F32R = dt.float32r

BS, N, J, C, K = 2, 4096, 3, 32, 32
NCORES = 8
NBLK = NCORES // BS          # 4 row-blocks per batch
NROWS = N // NBLK            # 1024 rows of W per core (the n-extent)
MT = N // 128                # 32 m-tiles (contraction)
ROW = J * NROWS              # 3072 elems per WT row
ACC_W = 512                  # psum bank width in fp32
NH = NROWS // ACC_W          # 2 output halves
# W DMA schedule: 14 double (2 m-tiles) + 3 single m-tile DMAs; the last
# m-tile goes as two half-column DMAs so its matmuls pipeline with arrival
DMA_TILES = [2] * 14 + [1] * 3

_CACHE: dict = {}


def _build_nc():
    nc = bacc.Bacc(None, target_bir_lowering=False)

    Wt = nc.dram_tensor("Wt", [N, ROW], F32R, kind="ExternalInput")
    XTt = nc.dram_tensor("XTt", [C, N], F32R, kind="ExternalInput")     # x^T
    CWTt = nc.dram_tensor("CWTt", [C, J * K], F32R, kind="ExternalInput")
    CBt = nc.dram_tensor("CBt", [K, 1], F32, kind="ExternalInput")
    Ot = nc.dram_tensor("Ot", [K, NROWS], F32, kind="ExternalOutput")

    with tile.TileContext(nc) as tc:
        with (
            tc.tile_pool(name="const", bufs=1) as constp,
            tc.tile_pool(name="wq", bufs=6) as wqp,
            tc.tile_pool(name="tp", bufs=5, space="PSUM") as tpp,
            tc.tile_pool(name="acc", bufs=1, space="PSUM") as accp,
            tc.tile_pool(name="outt", bufs=2) as outp,
        ):
            # --- tiny constant loads lead the sync ring (~2 us) while   ---
            # --- the scalar ring carries pure W from t=0; no SWDGE      ---
            # --- anywhere (its descriptor rings slow SDMA engines 7/15) ---
            XT = constp.tile([C, N], F32R)       # x^T: c on partitions
            nc.sync.dma_start(out=XT[:], in_=XTt[:, :])
            CWT = constp.tile([C, J * K], F32R)  # conv_w^T: c on partitions
            nc.sync.dma_start(out=CWT[:], in_=CWTt[:, :])
            CB = constp.tile([K, 1], F32)
            nc.sync.dma_start(out=CB[:], in_=CBt[:, :])

            # --- W DMAs fill both HWDGE rings back-to-back ---
            accs = [accp.tile([K, ACC_W], F32, name=f"acc{h}", tag=f"acc{h}")
                    for h in range(NH)]
            wts = []
            q0 = 0
            for s, ntile in enumerate(DMA_TILES):
                wt = wqp.tile([128, ntile * ROW], F32R, name=f"wq{s}", tag="wq")
                eng = nc.scalar if s % 2 == 0 else nc.sync
                eng.dma_start(
                    out=wt[:].rearrange("p (t v) -> p t v", t=ntile),
                    in_=Wt[
                        q0 * 128 : (q0 + ntile) * 128, :
                    ].rearrange("(t p) v -> p t v", p=128),
                )
                wts.append((wt, q0, ntile))
                q0 += ntile
            # last m-tile as two half-column DMAs on opposite rings
            HALF = ROW // 2
            tails = []
            for hs in range(2):
                wt = wqp.tile([128, HALF], F32R, name=f"wqt{hs}", tag="wq")
                eng = nc.scalar if hs == 0 else nc.sync
                eng.dma_start(
                    out=wt[:],
                    in_=Wt[q0 * 128 : (q0 + 1) * 128, hs * HALF : (hs + 1) * HALF],
                )
                tails.append(wt)

            Z = constp.tile([128, MT * J * K], F32R)  # z: m on partitions

            # --- Phase A: z[m, (j k)] = sum_c x[m,c] conv_w[k, j*C+c] ---
            for t in range(MT):
                pz = tpp.tile([128, J * K], F32, tag="tp")
                nc.tensor.matmul(
                    pz[:],
                    XT[:, t * 128 : (t + 1) * 128],
                    CWT[:],
                    start=True,
                    stop=True,
                )
                # fp32 -> f32r rounding copy
                nc.vector.tensor_copy(Z[:, t * J * K : (t + 1) * J * K], pz[:])

            # --- Phase B: main GEMM, W streamed once as the moving operand ---
            for wt, q0, ntile in wts:
                for tloc in range(ntile):
                    q = q0 + tloc
                    for j in range(J):
                        zq = Z[:, q * J * K + j * K : q * J * K + (j + 1) * K]
                        for h in range(NH):
                            base = tloc * ROW + j * NROWS + h * ACC_W
                            nc.tensor.matmul(
                                accs[h][:],
                                zq,
                                wt[:, base : base + ACC_W],
                                start=(q == 0 and j == 0),
                                stop=False,
                            )
            # last m-tile: consume each half as it lands; (j,h) slices of the
            # (j,n) row layout fall in half hs = (j*NROWS + h*ACC_W) // HALF
            q = MT - 1
            for j in range(J):
                zq = Z[:, q * J * K + j * K : q * J * K + (j + 1) * K]
                for h in range(NH):
                    off = j * NROWS + h * ACC_W
                    hs, base = divmod(off, HALF)
                    nc.tensor.matmul(
                        accs[h][:],
                        zq,
                        tails[hs][:, base : base + ACC_W],
                        start=False,
                        stop=(j == J - 1),
                    )

            # --- Phase C: bias + writeout ---
            for h in range(NH):
                ot = outp.tile([K, ACC_W], F32, name=f"ot{h}")
                nc.vector.tensor_scalar_add(ot[:], accs[h][:], CB[:, 0:1])
                nc.sync.dma_start(out=Ot[:, h * ACC_W : (h + 1) * ACC_W], in_=ot[:])

    nc.finalize()
    return nc


def _get_nc():
    if "nc" not in _CACHE:
        _CACHE["nc"] = _build_nc()
    return _CACHE["nc"]


def _relayout_w(Wslice):
    """[NROWS n, N m, J] -> [N m, J*NROWS (j,n)] blocked for host cache."""
    out = np.empty((N, J, NROWS), dtype=np.float32)
    B = 128
    for mb in range(0, N, B):
        blk = np.ascontiguousarray(Wslice[:, mb : mb + B, :])  # [NROWS, B, J]
        out[mb : mb + B] = blk.transpose(1, 2, 0)
    return out.reshape(N, ROW)


def _make_in_maps(W, x, conv_w, conv_b):
    cb = np.ascontiguousarray(conv_b.reshape(K, 1).astype(np.float32, copy=False))
    # CWT[c, j*K + k] = conv_w[k, j*C + c]
    cwt = np.ascontiguousarray(
        conv_w.astype(np.float32, copy=False).reshape(K, J, C).transpose(2, 1, 0)
    ).reshape(C, J * K)
    in_maps = []
    for core in range(NCORES):
        b, blk = divmod(core, NBLK)
        n0 = blk * NROWS
        in_maps.append(
            {
                "Wt": _relayout_w(W[b, n0 : n0 + NROWS]),
                "XTt": np.ascontiguousarray(x[b].T),
                "CWTt": cwt,
                "CBt": cb,
            }
        )
    return in_maps


def kernel(W, x, conv_w, conv_b, _trace=False, _trace_kwargs=None):
    nc = _get_nc()
    in_maps = _make_in_maps(W, x, conv_w, conv_b)
    r = run_bass_kernel_spmd(
        nc, in_maps, list(range(NCORES)), trace=_trace, **(_trace_kwargs or {})
    )
    out = np.empty((BS, K, N, 1), dtype=np.float32)
    for core in range(NCORES):
        b, blk = divmod(core, NBLK)
        n0 = blk * NROWS
        out[b, :, n0 : n0 + NROWS, 0] = r.results[core]["Ot"]
    _CACHE["last_result"] = r
    return out


# revision 14
# speedup vs baseline: 1.0473x; 1.0473x over previous
"""Trainium2 Bass kernel for nn_ConvLayer (gnn_message_passing).

Math: out[b,k,n] = sum_{m,j} W[b,n,m,j] * z_j[b,m,k] + bias[k]
  where z_j[b,m,k] = sum_c x[b,m,c] * conv_w[k, j*C+c]
(the reference's gmul + 1x1-conv collapses into one big GEMM against the
tiny precomputed z, streaming W exactly once -> memory-bound).

Sharding: 8 cores = 2 batches x 4 row-blocks of W's first node axis n.
Each core handles W[b, n0:n0+1024] (48 MiB), computes out[b, :, n0:n0+1024].
The shard handed to each core is laid out m-major on the host side
(WT[m, j*1024 + n] = W[b, n0+n, m, j]) so the contraction axis m lands on
SBUF partitions directly from the DMA; x / conv_w are likewise uploaded
pre-transposed (c on partitions).  These are pure permutations of the same
fp32 bytes -- all FLOPs (z build + main GEMM + bias) happen on-device and
W is still read from HBM in full fp32 exactly once.

Per-core device program (identical SPMD program, different data):
  Phase A: z build fully on-PE: pz[m-tile] = xT_tile^T @ cwT for all
    (j,k) at once -> Z (128, 32*96) f32r.  The tiny constant loads (xT,
    cwT, bias) lead the sync HWDGE ring while the scalar ring carries W
    from t=0.  No SWDGE (gpsimd) DMAs anywhere: any SWDGE use makes SDMA
    engines 7/15 run ~15% slow for the whole kernel, and the laggards'
    backlog then trickles out solo for ~20 us at the end.
  Phase B: stream WT in 14 DMAs of 3 MiB (2 m-tiles) + 3 of 1.5 MiB +
    2 half-tile tail DMAs, alternating the two HWDGE rings (scalar /
    sync); 12 KiB contiguous per (partition, m-tile) descriptors keep all
    16 SDMA engines at line rate (~425 GB/s/core aggregate -- the real
    per-core ceiling; both rings share the same 16 engines).  For each
    m-tile q and operator j, two accumulating f32r matmuls (stationary
    zq [128,32], moving WT slice [128,512]) add into the two PSUM
    accumulator banks.  W passes through the PE exactly once as the
    512-wide moving operand (~45 us PE vs ~123 us DMA -> purely
    DMA-bound).  The last m-tile arrives as two half-column DMAs on
    opposite rings so its matmuls pipeline with arrival.
  Phase C: bias + writeout (one output DMA per ring) -> DRAM as
    (32, 1024) = final (k, n) layout.
"""
import sys

if "/opt/trn_rl_repo" not in sys.path:
    sys.path.insert(0, "/opt/trn_rl_repo")

import numpy as np

import concourse.bacc as bacc
import concourse.mybir as mybir
from concourse import tile
from concourse.bass_utils import run_bass_kernel_spmd

dt = mybir.dt
F32 = dt.float32
F32R = dt.float32r

BS, N, J, C, K = 2, 4096, 3, 32, 32
NCORES = 8
NBLK = NCORES // BS          # 4 row-blocks per batch
NROWS = N // NBLK            # 1024 rows of W per core (the n-extent)
MT = N // 128                # 32 m-tiles (contraction)
ROW = J * NROWS              # 3072 elems per WT row
ACC_W = 512                  # psum bank width in fp32
NH = NROWS // ACC_W          # 2 output halves
# W DMA schedule: 14 double (2 m-tiles) + 3 single m-tile DMAs; the last
# m-tile goes as two half-column DMAs so its matmuls pipeline with arrival
DMA_TILES = [2] * 14 + [1] * 3

_CACHE: dict = {}


def _build_nc():
    nc = bacc.Bacc(None, target_bir_lowering=False)

    Wt = nc.dram_tensor("Wt", [N, ROW], F32R, kind="ExternalInput")
    XTt = nc.dram_tensor("XTt", [C, N], F32R, kind="ExternalInput")     # x^T
    CWTt = nc.dram_tensor("CWTt", [C, J * K], F32R, kind="ExternalInput")
    CBt = nc.dram_tensor("CBt", [K, 1], F32, kind="ExternalInput")
    Ot = nc.dram_tensor("Ot", [K, NROWS], F32, kind="ExternalOutput")

    with tile.TileContext(nc) as tc:
        with (
            tc.tile_pool(name="const", bufs=1) as constp,
            tc.tile_pool(name="wq", bufs=6) as wqp,
            tc.tile_pool(name="tp", bufs=5, space="PSUM") as tpp,
            tc.tile_pool(name="acc", bufs=1, space="PSUM") as accp,
            tc.tile_pool(name="outt", bufs=2) as outp,
        ):
            # --- tiny constant loads lead the sync ring (~2 us) while   ---
            # --- the scalar ring carries pure W from t=0; no SWDGE      ---
            # --- anywhere (its descriptor rings slow SDMA engines 7/15) ---
            XT = constp.tile([C, N], F32R)       # x^T: c on partitions
            nc.sync.dma_start(out=XT[:], in_=XTt[:, :])
            CWT = constp.tile([C, J * K], F32R)  # conv_w^T: c on partitions
            nc.sync.dma_start(out=CWT[:], in_=CWTt[:, :])
            CB = constp.tile([K, 1], F32)
            nc.sync.dma_start(out=CB[:], in_=CBt[:, :])

            # --- W DMAs fill both HWDGE rings back-to-back ---
            accs = [accp.tile([K, ACC_W], F32, name=f"acc{h}", tag=f"acc{h}")
                    for h in range(NH)]
            wts = []
            q0 = 0
            for s, ntile in enumerate(DMA_TILES):
                wt = wqp.tile([128, ntile * ROW], F32R, name=f"wq{s}", tag="wq")
                eng = nc.scalar if s % 2 == 0 else nc.sync
                eng.dma_start(
                    out=wt[:].rearrange("p (t v) -> p t v", t=ntile),
                    in_=Wt[
                        q0 * 128 : (q0 + ntile) * 128, :
                    ].rearrange("(t p) v -> p t v", p=128),
                )
                wts.append((wt, q0, ntile))
                q0 += ntile
            # last m-tile as two half-column DMAs on opposite rings
            HALF = ROW // 2
            tails = []
            for hs in range(2):
                wt = wqp.tile([128, HALF], F32R, name=f"wqt{hs}", tag="wq")
                eng = nc.scalar if hs == 0 else nc.sync
                eng.dma_start(
                    out=wt[:],
                    in_=Wt[q0 * 128 : (q0 + 1) * 128, hs * HALF : (hs + 1) * HALF],
                )
                tails.append(wt)

            Z = constp.tile([128, MT * J * K], F32R)  # z: m on partitions

            # --- Phase A: z[m, (j k)] = sum_c x[m,c] conv_w[k, j*C+c] ---
            for t in range(MT):
                pz = tpp.tile([128, J * K], F32, tag="tp")
                nc.tensor.matmul(
                    pz[:],
                    XT[:, t * 128 : (t + 1) * 128],
                    CWT[:],
                    start=True,
                    stop=True,
                )
                # fp32 -> f32r rounding copy
                nc.vector.tensor_copy(Z[:, t * J * K : (t + 1) * J * K], pz[:])

            # --- Phase B: main GEMM, W streamed once as the moving operand ---
            for wt, q0, ntile in wts:
                for tloc in range(ntile):
                    q = q0 + tloc
                    for j in range(J):
                        zq = Z[:, q * J * K + j * K : q * J * K + (j + 1) * K]
                        for h in range(NH):
                            base = tloc * ROW + j * NROWS + h * ACC_W
                            nc.tensor.matmul(
                                accs[h][:],
                                zq,
                                wt[:, base : base + ACC_W],
                                start=(q == 0 and j == 0),
                                stop=False,
                            )
            # last m-tile: consume each half as it lands; (j,h) slices of the
            # (j,n) row layout fall in half hs = (j*NROWS + h*ACC_W) // HALF
            q = MT - 1
            for j in range(J):
                zq = Z[:, q * J * K + j * K : q * J * K + (j + 1) * K]
                for h in range(NH):
                    off = j * NROWS + h * ACC_W
                    hs, base = divmod(off, HALF)
                    nc.tensor.matmul(
                        accs[h][:],
                        zq,
                        tails[hs][:, base : base + ACC_W],
                        start=False,
                        stop=(j == J - 1),
                    )

            # --- Phase C: bias + writeout ---
            for h in range(NH):
                ot = outp.tile([K, ACC_W], F32, name=f"ot{h}")
                nc.vector.tensor_scalar_add(ot[:], accs[h][:], CB[:, 0:1])
                eng = nc.scalar if h == 0 else nc.sync
                eng.dma_start(out=Ot[:, h * ACC_W : (h + 1) * ACC_W], in_=ot[:])

    nc.finalize()
    return nc


def _get_nc():
    if "nc" not in _CACHE:
        _CACHE["nc"] = _build_nc()
    return _CACHE["nc"]


def _relayout_w(Wslice):
    """[NROWS n, N m, J] -> [N m, J*NROWS (j,n)] blocked for host cache."""
    out = np.empty((N, J, NROWS), dtype=np.float32)
    B = 128
    for mb in range(0, N, B):
        blk = np.ascontiguousarray(Wslice[:, mb : mb + B, :])  # [NROWS, B, J]
        out[mb : mb + B] = blk.transpose(1, 2, 0)
    return out.reshape(N, ROW)


def _make_in_maps(W, x, conv_w, conv_b):
    cb = np.ascontiguousarray(conv_b.reshape(K, 1).astype(np.float32, copy=False))
    # CWT[c, j*K + k] = conv_w[k, j*C + c]
    cwt = np.ascontiguousarray(
        conv_w.astype(np.float32, copy=False).reshape(K, J, C).transpose(2, 1, 0)
    ).reshape(C, J * K)
    in_maps = []
    for core in range(NCORES):
        b, blk = divmod(core, NBLK)
        n0 = blk * NROWS
        in_maps.append(
            {
                "Wt": _relayout_w(W[b, n0 : n0 + NROWS]),
                "XTt": np.ascontiguousarray(x[b].T),
                "CWTt": cwt,
                "CBt": cb,
            }
        )
    return in_maps


def kernel(W, x, conv_w, conv_b, _trace=False, _trace_kwargs=None):
    nc = _get_nc()
    in_maps = _make_in_maps(W, x, conv_w, conv_b)
    r = run_bass_kernel_spmd(
        nc, in_maps, list(range(NCORES)), trace=_trace, **(_trace_kwargs or {})
    )
    out = np.empty((BS, K, N, 1), dtype=np.float32)
    for core in range(NCORES):
        b, blk = divmod(core, NBLK)
        n0 = blk * NROWS
        out[b, :, n0 : n0 + NROWS, 0] = r.results[core]["Ot"]
    _CACHE["last_result"] = r
    return out
